# revision 27
# baseline (speedup 1.0000x reference)
"""Trainium2 Bass kernel for a SimCLR-style contrastive loss (v7, fp8).

Math (per batch item b, xn/yn L2-normalized rows, tau = 0.01):
  x-row i logits = {S_xy[i, :]} u {S_xx[i, j != i]}    (2n-1 values)
  y-row j logits = {S_xy[:, j]} u {S_yy[j, i != j]}
  loss = mean over bs*2n rows of (logsumexp(logits) - S_xy[diag])

v7 = v6 scheduling + fp8e4 DoubleRow main matmuls:
  - scaled rows cast to fp8e4 (values ~N(0, 0.63), well inside +-240);
    numpy end-to-end check: loss rel err 7.9e-4 vs fp64 (gate is 2e-2).
  - operands stored [P, 2(k-chunk), 512] so one DoubleRow matmul contracts
    all of D=256: halves main-matmul PE time.
  - ALL four tensors are PE-transposed (XBAR can't move 1-byte dtypes).
    Transposes run in bf16 (walrus requires stride-2 packing for fp8
    transpose outputs); the PSUM->SBUF drain copy does the bf16->fp8
    cast for free.  Item-0 drains on ScalarE (idle prologue), item-1
    drains on DVE.
  - exp/colsum path stays bf16: ee tiles, ones-matmul colsums, rowsum
    accum, diag -1e5 mask.
  - colsum vectors accumulate in a persistent [P, N] fp32 PSUM tile rows
    {0,32,64}; drained via one [1, N] DVE copy + 8 tiny K=1 PE transposes
    into a persistent [P, 512] PSUM tile (no DMA, no pool-slot waits).
  - output store on the Sync HWDGE queue (cheap teardown).
"""

from contextlib import ExitStack

import numpy as np

import concourse.bacc as bacc
import concourse.tile as tile
from concourse import mybir
from concourse.bass_utils import run_bass_kernel_spmd

BS, N, D = 16, 1024, 256
NCORES = 8
IPC = BS // NCORES  # items per core
P = 128
NT = N // P  # 128-row blocks per item
KC = D // P  # contraction chunks
HB = 512  # one PSUM bank of fp32
NEG = -100000.0  # folded into S_xx/S_yy diag -> exp() == 0.0
LN10 = 2.302585092994046

dt = mybir.dt
AF = mybir.ActivationFunctionType
ALU = mybir.AluOpType
AX = mybir.AxisListType
PM = mybir.MatmulPerfMode
F32 = dt.float32
BF16 = dt.bfloat16
FP8 = dt.float8e4


def _pin_act_table(nc):
    """Emit an explicit table load for the set containing BOTH Exp and Ln,
    so bacc's fixpoint pass never needs to swap tables mid-kernel."""
    from concourse.hw_specs import get_activation_tables

    tabs = list(get_activation_tables(nc.m.arch).items())
    setid = next(i for i, (_, fns) in enumerate(tabs)
                 if AF.Exp in fns and AF.Ln in fns)
    nc.scalar.add_instruction(mybir.InstLoadActFuncSet(
        name=nc.get_next_instruction_name(), ins=[], outs=[],
        act_func_set_id=setid))


def build_nc():
    nc = bacc.Bacc("TRN2", target_bir_lowering=False, debug=False)

    x_in = nc.dram_tensor("x", [IPC * N, D], F32, kind="ExternalInput")
    y_in = nc.dram_tensor("y", [IPC * N, D], F32, kind="ExternalInput")
    idt_in = nc.dram_tensor("idt", [P, P], BF16, kind="ExternalInput")
    negid_in = nc.dram_tensor("negid", [P, P], BF16, kind="ExternalInput")
    out_d = nc.dram_tensor("out", [P, 3 * IPC], F32, kind="ExternalOutput")

    with tile.TileContext(nc) as tc, ExitStack() as ctx:
        const = ctx.enter_context(tc.tile_pool(name="const", bufs=1))
        nat = ctx.enter_context(tc.tile_pool(name="nat", bufs=2))
        xbp = ctx.enter_context(tc.tile_pool(name="xbp", bufs=2))
        opT = ctx.enter_context(tc.tile_pool(name="opT", bufs=2))
        eep = ctx.enter_context(tc.tile_pool(name="eep", bufs=10))
        stat = ctx.enter_context(tc.tile_pool(name="stat", bufs=2))
        scr = ctx.enter_context(tc.tile_pool(name="scr", bufs=3))
        cssb = ctx.enter_context(tc.tile_pool(name="cssb", bufs=2))
        finp = ctx.enter_context(tc.tile_pool(name="finp", bufs=1))
        dram = ctx.enter_context(tc.tile_pool(name="dram", bufs=2, space="DRAM"))
        ps2 = ctx.enter_context(tc.tile_pool(name="ps2", bufs=2, space="PSUM"))
        ps1 = ctx.enter_context(tc.tile_pool(name="ps1", bufs=1, space="PSUM"))
        psc = ctx.enter_context(tc.tile_pool(name="psc", bufs=1, space="PSUM"))

        _pin_act_table(nc)

        # input loads on the gpsimd software-DGE queue: cheap to issue, on
        # separate rings, never block the Sync HWDGE chain.
        def load_nat(tname, it, src, order=(0, 1), quarters=False, eng=None):
            t = nat.tile([P, NT * D], F32, tag=f"nat{tname}", name=f"nat{tname}{it}")
            Q = NT // 4 if quarters else NT // 2
            hs = [q for h in order for q in ((2 * h, 2 * h + 1) if quarters
                                            else (h,))]
            for h in hs:
                (eng or nc.gpsimd).dma_start(
                    t[:, h * Q * D:(h + 1) * Q * D].rearrange(
                        "p (m d) -> p m d", m=Q),
                    src[it * N + h * Q * P:it * N + (h + 1) * Q * P, :]
                    .rearrange("(m p) d -> p m d", p=P))
            return t

        # item-0 loads on the Sync HWDGE queue: SWDGE descriptor generation
        # costs ~1us per strided load and was gating the whole prologue
        nx0 = load_nat("x", 0, x_in, order=(1, 0), quarters=True, eng=nc.sync)
        ny0 = load_nat("y", 0, y_in, order=(1, 0), eng=nc.sync)
        nx1 = load_nat("x", 1, x_in)
        ny1 = load_nat("y", 1, y_in)

        idt = const.tile([P, P], BF16, tag="idt")
        nc.sync.dma_start(idt[:], idt_in[:])
        negid = const.tile([P, P], BF16, tag="negid")
        nc.sync.dma_start(negid[:], negid_in[:])
        ones_b = const.tile([P, 1], BF16, tag="ones_b")
        nc.vector.memset(ones_b[:], 1.0)
        zeros_b = const.tile([P, 1], BF16, tag="zeros_b")
        nc.vector.memset(zeros_b[:], 0.0)
        ones_f = const.tile([P, 1], F32, tag="ones_f")
        nc.vector.memset(ones_f[:], 1.0)
        ln10c = const.tile([P, 1], F32, tag="ln10c")
        nc.vector.memset(ln10c[:], LN10)

        # fin columns per item: [sum ln Tx, sum ln Ty, -2*pos_sum]
        fin = finp.tile([P, 3 * IPC], F32, tag="fin")

        # one persistent [P, N] fp32 PSUM tile (2 banks) holds every phase's
        # colsum vector on its own partition row (item 1 reuses item 0's
        # rows after their drains); a second persistent [P, HB] tile holds
        # the six transposed [P, NT] colsum blocks.
        cs_all = psc.tile([P, N], F32, tag="cs_all")
        CS_ROW = {"xx0": 0, "xy0": 32, "yy0": 64,
                  "xx1": 0, "xy1": 32, "yy1": 64}
        csp_all = psc.tile([P, HB], F32, tag="csp_all")
        CSP_IDX = {"xx0": 0, "xy0": 1, "yy0": 2, "xx1": 3, "xy1": 4, "yy1": 5}

        # HAM warm-up: fp32 N=512 matmuls gated only on the first input half,
        # filling the otherwise-idle PE window during prep. PE transposes
        # don't count as HAM activity, so without these the transposes AND
        # the first real matmuls run at 1.2 GHz.
        WOFF = (NT // 2) * D
        warm_ps = ps2.tile([1, HB], F32, tag="ps2", name="warm_ps")
        for w in range(8):
            nc.tensor.matmul(warm_ps[:], ones_f[:], nx0[:, WOFF:WOFF + HB],
                             start=True, stop=True)

        def prep_arith(tname, it, nt_, h, ss, inv10, b):
            """ssq + 10/||row|| + scale-to-fp8 for one 512-row half."""
            H = NT // 2
            for mt in range(h * H, (h + 1) * H):
                sq = scr.tile([P, D], BF16, tag="sq", name="sq")
                nc.vector.scalar_tensor_tensor(
                    sq[:], nt_[:, mt * D:(mt + 1) * D], 1.0,
                    nt_[:, mt * D:(mt + 1) * D], ALU.mult, ALU.mult,
                    accum_out=ss[:, mt:mt + 1])
            lns = scr.tile([P, NT], F32, tag="lns", name="lns")
            nc.scalar.activation(lns[:, h * H:(h + 1) * H],
                                 ss[:, h * H:(h + 1) * H], AF.Ln)
            nc.scalar.activation(inv10[:, h * H:(h + 1) * H],
                                 lns[:, h * H:(h + 1) * H], AF.Exp,
                                 scale=-0.5, bias=ln10c[:])
            for mt in range(h * H, (h + 1) * H):
                nc.vector.tensor_scalar(
                    b[:, mt * D:(mt + 1) * D], nt_[:, mt * D:(mt + 1) * D],
                    inv10[:, mt:mt + 1], None, ALU.mult)

        def prep_pe_T(tname, it, b, h, t8, drain):
            """PE transposes for one half into the [P, KC, N] operand tile:
            4 blocks per k-chunk via PSUM (bf16), drain casts to fp8."""
            H = NT // 2
            for k in range(KC):
                tp = ps1.tile([P, HB], BF16, tag="ps1", name="tp")
                for i, mt in enumerate(range(h * H, (h + 1) * H)):
                    nc.tensor.transpose(
                        tp[:, i * P:(i + 1) * P],
                        b[:, mt * D + k * P:mt * D + (k + 1) * P], idt[:])
                drain(t8[:, k, h * HB:(h + 1) * HB], tp[:])

        def mk_op(tname, it):
            return opT.tile([P, KC, N], FP8, tag=f"{tname}T8",
                            name=f"{tname}T8_{it}")

        def drain_cs(vtag):
            """cs row [1, N] -> SBUF staging (DVE) -> 8 tiny K=1 PE
            transposes -> [P, NT] slice of csp_all.  Zero DMA involvement."""
            row = CS_ROW[vtag]
            sb = cssb.tile([1, N], F32, tag="cs_sb", name=f"sb_{vtag}")
            for nh in range(2):
                nc.vector.tensor_copy(sb[:, nh * HB:(nh + 1) * HB],
                                      cs_all[row:row + 1, nh * HB:(nh + 1) * HB])
            csp = csp_all[:, CSP_IDX[vtag] * NT:(CSP_IDX[vtag] + 1) * NT]
            for j in range(NT):
                nc.tensor.transpose(csp[:, j:j + 1],
                                    sb[0:1, j * P:(j + 1) * P],
                                    ones_f[0:1, 0:1])
            return csp

        # sym-phase row-block order: alternate between the ps1 pool (mt>=4,
        # one bank) and ps2 (mt<4, two banks) so multiple row-blocks are in
        # flight and the PE stream stays dense.
        SYM_ORDER = [4, 0, 5, 1, 6, 2, 3, 7]
        # last contributing mt per colsum half, in emission order
        _contrib = {nh: [mt for mt in SYM_ORDER
                         if max(mt * P + P, nh * HB) < min((nh + 1) * HB, N)]
                    for nh in range(2)}

        def sym_phase(oT, vtag, bridge, warm=0, last=False):
            """Upper-triangle similarity phase: returns rowsums [P, NT]."""
            row = CS_ROW[vtag]
            cs = cs_all[row:row + 1, :]
            rs = stat.tile([P, NT], F32, tag=f"rs{vtag[:2]}", name=f"rs{vtag}")
            # cols [0, P) are never written by any strictly-upper colsum but
            # ARE read by the drain: one tiny zeroing matmul covers them.
            # All other columns get start=True on their first contributor
            # (tracked via cover), so the full-row zeroing openers are gone.
            nc.tensor.matmul(cs[:, 0:P], zeros_b[:], bridge[:, 0:P],
                             start=True, stop=True)
            for w in range(warm):
                nc.tensor.matmul(warm_ps[:], zeros_b[:], bridge[:, 0:HB],
                                 start=True, stop=True)
            pending = []  # colsum matmuls deferred by a few strips

            def flush(n):
                while len(pending) > n:
                    for (dst, lhs, src, start, stop) in pending.pop(0):
                        nc.tensor.matmul(dst, lhs, src, start=start, stop=stop)

            for mt in SYM_ORDER:
                lo = mt * P
                if lo < HB:
                    ps = ps2.tile([P, N], F32, tag="ps2", name="ps_sym")
                    base = 0
                    chunks = [(lo, HB), (HB, N)]
                else:
                    ps = ps1.tile([P, HB], F32, tag="ps1", name="ps_sym1")
                    base = HB
                    chunks = [(lo, N)]
                for ci, (c0, c1) in enumerate(chunks):
                    nc.tensor.matmul(
                        ps[:, c0 - base:c1 - base],
                        oT[:, :, mt * P:(mt + 1) * P],
                        oT[:, :, c0:c1],
                        start=True, stop=(ci > 0),
                        perf_mode=PM.DoubleRow)
                # diag mask: add -1e5*I to [lo, lo+P) inside the group
                nc.tensor.matmul(
                    ps[:, lo - base:lo - base + P], idt[:], negid[:],
                    start=False, stop=True)
                flush(2)
                ee = eep.tile([P, N], BF16, tag="ee", name="ee_sym")
                if N - lo <= 384:
                    # ACT's accum costs a fixed 187ns read; a narrow DVE
                    # reduce is cheaper and ACT is the bottleneck engine
                    nc.scalar.activation(ee[:, lo:], ps[:, lo - base:], AF.Exp)
                    nc.vector.reduce_sum(rs[:, mt:mt + 1], ee[:, lo:],
                                         axis=AX.X)
                else:
                    nc.scalar.activation(ee[:, lo:], ps[:, lo - base:],
                                         AF.Exp, accum_out=rs[:, mt:mt + 1])
                # strictly-upper colsums (lower-triangle rowsums by symmetry)
                # start=True marks the whole per-partition 2KB zero region
                # pending and a write may not straddle the pending/cleared
                # boundary, so: the first emitted strip (mt 4) opens bank 1
                # clearing [640, 1024); strip 0 opens bank 0 and splits its
                # bank-1 write at 640 (left piece overwrites pending bytes,
                # right piece accumulates); everything later is start=False
                # over already-cleared bytes.
                grp = []
                for nh in range(2):
                    a = max(lo + P, nh * HB)
                    b = min((nh + 1) * HB, N)
                    if a >= b:
                        continue
                    stop = mt == _contrib[nh][-1]
                    sp = SYM_ORDER[0] * P + P
                    if mt == 0 and nh == 1 and a < sp < b:
                        grp.append((cs[:, a:sp], ones_b[:], ee[:, a:sp],
                                    False, stop))
                        grp.append((cs[:, sp:b], ones_b[:], ee[:, sp:b],
                                    False, stop))
                    else:
                        start = (mt == SYM_ORDER[0] and nh == 1) or (
                            mt == 0 and nh == 0)
                        grp.append((cs[:, a:b], ones_b[:], ee[:, a:b],
                                    start, stop))
                pending.append(grp)
                # tail phase: emit every pending colsum as soon as the last
                # contributor's exp is queued, so the cs drain chain (copy ->
                # transposes -> ln -> out) starts ~1.5us earlier
                if last and mt == _contrib[1][-1]:
                    flush(0)
            flush(0)
            return rs

        def xy_phase(vtag, xT, yT, bridge):
            row = CS_ROW[vtag]
            cs = cs_all[row:row + 1, :]
            rs = stat.tile([P, NT], F32, tag="rs_xy", name=f"rs_{vtag}")
            pending = []

            def flush(n):
                while len(pending) > n:
                    for (dst, lhs, src, start, stop) in pending.pop(0):
                        nc.tensor.matmul(dst, lhs, src, start=start, stop=stop)

            for mt in range(NT):
                ps = ps2.tile([P, N], F32, tag="ps2", name="ps_xy")
                for nh in range(2):
                    nc.tensor.matmul(
                        ps[:, nh * HB:(nh + 1) * HB],
                        xT[:, :, mt * P:(mt + 1) * P],
                        yT[:, :, nh * HB:(nh + 1) * HB],
                        start=True, stop=True,
                        perf_mode=PM.DoubleRow)
                flush(2)
                ee = eep.tile([P, N], BF16, tag="ee", name="ee_xy")
                nc.scalar.activation(ee[:], ps[:], AF.Exp,
                                     accum_out=rs[:, mt:mt + 1])
                pending.append([
                    (cs[:, nh * HB:(nh + 1) * HB], ones_b[:],
                     ee[:, nh * HB:(nh + 1) * HB], mt == 0, mt == NT - 1)
                    for nh in range(2)])
            flush(0)
            return rs

        def pos_diag(it, xb, yb):
            pos = stat.tile([P, NT], F32, tag="pos", name=f"pos{it}")
            for mt in range(NT):
                pq = scr.tile([P, D], BF16, tag="pq", name="pq")
                nc.vector.scalar_tensor_tensor(
                    pq[:], xb[:, mt * D:(mt + 1) * D], 1.0,
                    yb[:, mt * D:(mt + 1) * D], ALU.mult, ALU.mult,
                    accum_out=pos[:, mt:mt + 1])
            return pos

        def combine_x(it, rs_xx, csT_xx, rs_xy):
            tx = stat.tile([P, NT], F32, tag="tx", name=f"tx{it}")
            nc.vector.tensor_add(tx[:], rs_xy[:], rs_xx[:])
            nc.vector.tensor_add(tx[:], tx[:], csT_xx[:])
            lnx = scr.tile([P, NT], F32, tag="lnx", name="lnx")
            nc.scalar.activation(lnx[:], tx[:], AF.Ln,
                                 accum_out=fin[:, 3 * it:3 * it + 1])

        def combine_y(it, rs_yy, csT_xy, csT_yy):
            ty = stat.tile([P, NT], F32, tag="ty", name=f"ty{it}")
            nc.vector.tensor_add(ty[:], rs_yy[:], csT_xy[:])
            nc.vector.tensor_add(ty[:], ty[:], csT_yy[:])
            lny = scr.tile([P, NT], F32, tag="lny", name="lny")
            nc.scalar.activation(lny[:], ty[:], AF.Ln,
                                 accum_out=fin[:, 3 * it + 1:3 * it + 2])

        def combine_p(it, pos):
            posr = stat.tile([P, 1], F32, tag="posr", name=f"posr{it}")
            nc.vector.reduce_sum(posr[:], pos[:], axis=AX.X)
            nc.vector.tensor_scalar_mul(fin[:, 3 * it + 2:3 * it + 3],
                                        posr[:], -2.0)

        drainA = lambda dst, src: nc.scalar.copy(dst, src)
        drainV = lambda dst, src: nc.vector.tensor_copy(dst, src)

        # ---- item-0 prep: x fully first (xx0 gates on it), y's arithmetic
        # before xx0 (runs on DVE/ACT during xx0), y's PE transposes after
        # xx0's matmuls in the PE FIFO.
        ssx0 = stat.tile([P, NT], F32, tag="ssx", name="ssx0")
        invx0 = stat.tile([P, NT], F32, tag="invx", name="invx0")
        xb0 = xbp.tile([P, NT * D], BF16, tag="xb", name="xb0")
        xT0 = mk_op("x", 0)
        ssy0 = stat.tile([P, NT], F32, tag="ssy", name="ssy0")
        invy0 = stat.tile([P, NT], F32, tag="invy", name="invy0")
        yb0 = xbp.tile([P, NT * D], BF16, tag="yb", name="yb0")
        yT0 = mk_op("y", 0)
        for h in (1, 0):
            prep_arith("x", 0, nx0, h, ssx0, invx0, xb0)
            prep_pe_T("x", 0, xb0, h, xT0, drainA if h == 1 else drainV)
        for h in (1, 0):
            prep_arith("y", 0, ny0, h, ssy0, invy0, yb0)

        # ---- software-pipelined phase emission
        rs_xx0 = sym_phase(xT0, "xx0", xb0, warm=4)
        for h in (1, 0):
            prep_pe_T("y", 0, yb0, h, yT0, drainV)

        rs_xy0 = xy_phase("xy0", xT0, yT0, xb0)

        # item-1 prep: arithmetic + PE transposes (drained on DVE; ScalarE
        # is the busy engine mid-kernel)
        def prep_full(tname, it, nt_, b, ts):
            """item-1 prep: one merged norm chain."""
            ss = stat.tile([P, NT], F32, tag=f"ss{tname}", name=f"ss{tname}{it}")
            inv10 = stat.tile([P, NT], F32, tag=f"inv{tname}",
                              name=f"inv{tname}{it}")
            for mt in range(NT):
                sq = scr.tile([P, D], BF16, tag="sq", name="sq")
                nc.vector.scalar_tensor_tensor(
                    sq[:], nt_[:, mt * D:(mt + 1) * D], 1.0,
                    nt_[:, mt * D:(mt + 1) * D], ALU.mult, ALU.mult,
                    accum_out=ss[:, mt:mt + 1])
            lns = scr.tile([P, NT], F32, tag="lns", name="lns")
            nc.scalar.activation(lns[:], ss[:], AF.Ln)
            nc.scalar.activation(inv10[:], lns[:], AF.Exp,
                                 scale=-0.5, bias=ln10c[:])
            for mt in range(NT):
                nc.vector.tensor_scalar(
                    b[:, mt * D:(mt + 1) * D], nt_[:, mt * D:(mt + 1) * D],
                    inv10[:, mt:mt + 1], None, ALU.mult)
            # DMA-XBAR transposes (bf16, Sync HWDGE) + one [P, N] fp8
            # cast per k-chunk on DVE: keeps the PE free for phase matmuls
            bd = dram.tile([N, D], BF16, tag=f"{tname}bd", name=f"{tname}bd{it}")
            nc.sync.dma_start(
                bd[:].rearrange("(m p) d -> p m d", p=P),
                b[:].rearrange("p (m d) -> p m d", m=NT))
            for k in range(KC):
                tb = opT.tile([P, N], BF16, tag=f"{tname}Tb{k}",
                              name=f"{tname}Tb{k}_{it}")
                nc.sync.dma_start_transpose(tb[:], bd[:, k * P:(k + 1) * P])
                nc.vector.tensor_copy(ts[:, k, :], tb[:])

        xb1 = xbp.tile([P, NT * D], BF16, tag="xb", name="xb1")
        xT1 = mk_op("x", 1)
        prep_full("x", 1, nx1, xb1, xT1)
        csT_xx0 = drain_cs("xx0")

        yb1 = xbp.tile([P, NT * D], BF16, tag="yb", name="yb1")
        yT1 = mk_op("y", 1)
        prep_full("y", 1, ny1, yb1, yT1)
        csT_xy0 = drain_cs("xy0")

        rs_yy0 = sym_phase(yT0, "yy0", yb0)
        csT_yy0 = drain_cs("yy0")
        pos0 = pos_diag(0, xb0, yb0)

        rs_xx1 = sym_phase(xT1, "xx1", xb1)
        csT_xx1 = drain_cs("xx1")
        combine_x(0, rs_xx0, csT_xx0, rs_xy0)
        combine_y(0, rs_yy0, csT_xy0, csT_yy0)
        combine_p(0, pos0)

        rs_xy1 = xy_phase("xy1", xT1, yT1, xb1)
        csT_xy1 = drain_cs("xy1")
        pos1 = pos_diag(1, xb1, yb1)
        combine_p(1, pos1)

        rs_yy1 = sym_phase(yT1, "yy1", yb1, last=True)
        combine_x(1, rs_xx1, csT_xx1, rs_xy1)
        csT_yy1 = drain_cs("yy1")
        combine_y(1, rs_yy1, csT_xy1, csT_yy1)

        # ship the [128, 6] partials raw; the host does the final reduction.
        # Sync HWDGE: its queue is idle by now and its teardown is ~ns,
        # where the gpsimd SWDGE drain cost ~2.5us after the last op.
        nc.sync.dma_start(out_d[:], fin[:])

    nc.compile()
    return nc


_CACHE = {}
TRACE = False
LAST_RESULTS = None


def _get_nc():
    if "nc" not in _CACHE:
        _CACHE["nc"] = build_nc()
    return _CACHE["nc"]


def make_in_maps(x, y):
    import ml_dtypes

    x = np.ascontiguousarray(np.asarray(x, dtype=np.float32))
    y = np.ascontiguousarray(np.asarray(y, dtype=np.float32))
    idt = np.eye(P, dtype=np.float32).astype(ml_dtypes.bfloat16)
    negid = (np.eye(P, dtype=np.float32) * NEG).astype(ml_dtypes.bfloat16)
    in_maps = []
    for c in range(NCORES):
        in_maps.append({
            "x": x[c * IPC:(c + 1) * IPC].reshape(IPC * N, D),
            "y": y[c * IPC:(c + 1) * IPC].reshape(IPC * N, D),
            "idt": idt,
            "negid": negid,
        })
    return in_maps


def kernel(x, y):
    global LAST_RESULTS
    nc = _get_nc()
    in_maps = make_in_maps(x, y)
    res = run_bass_kernel_spmd(nc, in_maps, list(range(NCORES)), trace=TRACE)
    LAST_RESULTS = res
    partials = np.array([np.asarray(r["out"], dtype=np.float64).sum()
                         for r in res.results])
    return np.float32(partials.sum() / (BS * 2 * N))


# revision 28
# speedup vs baseline: 1.1788x; 1.1788x over previous
"""Trainium2 Bass kernel for a SimCLR-style contrastive loss (v7, fp8).

Math (per batch item b, xn/yn L2-normalized rows, tau = 0.01):
  x-row i logits = {S_xy[i, :]} u {S_xx[i, j != i]}    (2n-1 values)
  y-row j logits = {S_xy[:, j]} u {S_yy[j, i != j]}
  loss = mean over bs*2n rows of (logsumexp(logits) - S_xy[diag])

v7 = v6 scheduling + fp8e4 DoubleRow main matmuls:
  - scaled rows cast to fp8e4 (values ~N(0, 0.63), well inside +-240);
    numpy end-to-end check: loss rel err 7.9e-4 vs fp64 (gate is 2e-2).
  - operands stored [P, 2(k-chunk), 512] so one DoubleRow matmul contracts
    all of D=256: halves main-matmul PE time.
  - ALL four tensors are PE-transposed (XBAR can't move 1-byte dtypes).
    Transposes run in bf16 (walrus requires stride-2 packing for fp8
    transpose outputs); the PSUM->SBUF drain copy does the bf16->fp8
    cast for free.  Item-0 drains on ScalarE (idle prologue), item-1
    drains on DVE.
  - exp/colsum path stays bf16: ee tiles, ones-matmul colsums, rowsum
    accum, diag -1e5 mask.
  - colsum vectors accumulate in a persistent [P, N] fp32 PSUM tile rows
    {0,32,64}; drained via one [1, N] DVE copy + 8 tiny K=1 PE transposes
    into a persistent [P, 512] PSUM tile (no DMA, no pool-slot waits).
  - output store on the Sync HWDGE queue (cheap teardown).
"""

from contextlib import ExitStack

import numpy as np

import concourse.bacc as bacc
import concourse.tile as tile
from concourse import mybir
from concourse.bass_utils import run_bass_kernel_spmd

BS, N, D = 16, 1024, 256
NCORES = 8
IPC = BS // NCORES  # items per core
P = 128
NT = N // P  # 128-row blocks per item
KC = D // P  # contraction chunks
HB = 512  # one PSUM bank of fp32
NEG = -100000.0  # folded into S_xx/S_yy diag -> exp() == 0.0
LN10 = 2.302585092994046

dt = mybir.dt
AF = mybir.ActivationFunctionType
ALU = mybir.AluOpType
AX = mybir.AxisListType
PM = mybir.MatmulPerfMode
F32 = dt.float32
BF16 = dt.bfloat16
FP8 = dt.float8e4


def _pin_act_table(nc):
    """Emit an explicit table load for the set containing BOTH Exp and Ln,
    so bacc's fixpoint pass never needs to swap tables mid-kernel."""
    from concourse.hw_specs import get_activation_tables

    tabs = list(get_activation_tables(nc.m.arch).items())
    setid = next(i for i, (_, fns) in enumerate(tabs)
                 if AF.Exp in fns and AF.Ln in fns)
    nc.scalar.add_instruction(mybir.InstLoadActFuncSet(
        name=nc.get_next_instruction_name(), ins=[], outs=[],
        act_func_set_id=setid))


def build_nc():
    nc = bacc.Bacc("TRN2", target_bir_lowering=False, debug=False)

    x_in = nc.dram_tensor("x", [IPC * N, D], F32, kind="ExternalInput")
    y_in = nc.dram_tensor("y", [IPC * N, D], F32, kind="ExternalInput")
    idt_in = nc.dram_tensor("idt", [P, P], BF16, kind="ExternalInput")
    negid_in = nc.dram_tensor("negid", [P, P], BF16, kind="ExternalInput")
    out_d = nc.dram_tensor("out", [P, 3 * IPC], F32, kind="ExternalOutput")

    with tile.TileContext(nc) as tc, ExitStack() as ctx:
        const = ctx.enter_context(tc.tile_pool(name="const", bufs=1))
        nat = ctx.enter_context(tc.tile_pool(name="nat", bufs=2))
        xbp = ctx.enter_context(tc.tile_pool(name="xbp", bufs=2))
        opT = ctx.enter_context(tc.tile_pool(name="opT", bufs=2))
        eep = ctx.enter_context(tc.tile_pool(name="eep", bufs=10))
        stat = ctx.enter_context(tc.tile_pool(name="stat", bufs=2))
        scr = ctx.enter_context(tc.tile_pool(name="scr", bufs=3))
        cssb = ctx.enter_context(tc.tile_pool(name="cssb", bufs=2))
        finp = ctx.enter_context(tc.tile_pool(name="finp", bufs=1))
        dram = ctx.enter_context(tc.tile_pool(name="dram", bufs=2, space="DRAM"))
        ps2 = ctx.enter_context(tc.tile_pool(name="ps2", bufs=2, space="PSUM"))
        ps1 = ctx.enter_context(tc.tile_pool(name="ps1", bufs=1, space="PSUM"))
        psc = ctx.enter_context(tc.tile_pool(name="psc", bufs=1, space="PSUM"))

        _pin_act_table(nc)

        # input loads on the gpsimd software-DGE queue: cheap to issue, on
        # separate rings, never block the Sync HWDGE chain.
        def load_nat(tname, it, src, order=(0, 1), quarters=False):
            t = nat.tile([P, NT * D], F32, tag=f"nat{tname}", name=f"nat{tname}{it}")
            Q = NT // 4 if quarters else NT // 2
            hs = [q for h in order for q in ((2 * h, 2 * h + 1) if quarters
                                            else (h,))]
            for h in hs:
                nc.gpsimd.dma_start(
                    t[:, h * Q * D:(h + 1) * Q * D].rearrange(
                        "p (m d) -> p m d", m=Q),
                    src[it * N + h * Q * P:it * N + (h + 1) * Q * P, :]
                    .rearrange("(m p) d -> p m d", p=P))
            return t

        nx0 = load_nat("x", 0, x_in, order=(1, 0), quarters=True)
        ny0 = load_nat("y", 0, y_in)
        nx1 = load_nat("x", 1, x_in)
        ny1 = load_nat("y", 1, y_in)

        idt = const.tile([P, P], BF16, tag="idt")
        nc.sync.dma_start(idt[:], idt_in[:])
        negid = const.tile([P, P], BF16, tag="negid")
        nc.sync.dma_start(negid[:], negid_in[:])
        ones_b = const.tile([P, 1], BF16, tag="ones_b")
        nc.vector.memset(ones_b[:], 1.0)
        zeros_b = const.tile([P, 1], BF16, tag="zeros_b")
        nc.vector.memset(zeros_b[:], 0.0)
        ones_f = const.tile([P, 1], F32, tag="ones_f")
        nc.vector.memset(ones_f[:], 1.0)
        ln10c = const.tile([P, 1], F32, tag="ln10c")
        nc.vector.memset(ln10c[:], LN10)

        # fin columns per item: [sum ln Tx, sum ln Ty, -2*pos_sum]
        fin = finp.tile([P, 3 * IPC], F32, tag="fin")

        # one persistent [P, N] fp32 PSUM tile (2 banks) holds every phase's
        # colsum vector on its own partition row (item 1 reuses item 0's
        # rows after their drains); a second persistent [P, HB] tile holds
        # the six transposed [P, NT] colsum blocks.
        cs_all = psc.tile([P, N], F32, tag="cs_all")
        CS_ROW = {"xx0": 0, "xy0": 32, "yy0": 64,
                  "xx1": 0, "xy1": 32, "yy1": 64}
        csp_all = psc.tile([P, HB], F32, tag="csp_all")
        CSP_IDX = {"xx0": 0, "xy0": 1, "yy0": 2, "xx1": 3, "xy1": 4, "yy1": 5}

        # HAM warm-up: fp32 N=512 matmuls gated only on the first input half,
        # filling the otherwise-idle PE window during prep. PE transposes
        # don't count as HAM activity, so without these the transposes AND
        # the first real matmuls run at 1.2 GHz.
        WOFF = (NT // 2) * D
        warm_ps = ps2.tile([1, HB], F32, tag="ps2", name="warm_ps")
        for w in range(8):
            nc.tensor.matmul(warm_ps[:], ones_f[:], nx0[:, WOFF:WOFF + HB],
                             start=True, stop=True)

        def prep_arith(tname, it, nt_, h, ss, inv10, b):
            """ssq + 10/||row|| + scale-to-fp8 for one 512-row half."""
            H = NT // 2
            for mt in range(h * H, (h + 1) * H):
                sq = scr.tile([P, D], BF16, tag="sq", name="sq")
                nc.vector.scalar_tensor_tensor(
                    sq[:], nt_[:, mt * D:(mt + 1) * D], 1.0,
                    nt_[:, mt * D:(mt + 1) * D], ALU.mult, ALU.mult,
                    accum_out=ss[:, mt:mt + 1])
            lns = scr.tile([P, NT], F32, tag="lns", name="lns")
            nc.scalar.activation(lns[:, h * H:(h + 1) * H],
                                 ss[:, h * H:(h + 1) * H], AF.Ln)
            nc.scalar.activation(inv10[:, h * H:(h + 1) * H],
                                 lns[:, h * H:(h + 1) * H], AF.Exp,
                                 scale=-0.5, bias=ln10c[:])
            for mt in range(h * H, (h + 1) * H):
                nc.vector.tensor_scalar(
                    b[:, mt * D:(mt + 1) * D], nt_[:, mt * D:(mt + 1) * D],
                    inv10[:, mt:mt + 1], None, ALU.mult)

        def prep_pe_T(tname, it, b, h, t8, drain):
            """PE transposes for one half into the [P, KC, N] operand tile:
            4 blocks per k-chunk via PSUM (bf16), drain casts to fp8."""
            H = NT // 2
            for k in range(KC):
                tp = ps1.tile([P, HB], BF16, tag="ps1", name="tp")
                for i, mt in enumerate(range(h * H, (h + 1) * H)):
                    nc.tensor.transpose(
                        tp[:, i * P:(i + 1) * P],
                        b[:, mt * D + k * P:mt * D + (k + 1) * P], idt[:])
                drain(t8[:, k, h * HB:(h + 1) * HB], tp[:])

        def mk_op(tname, it):
            return opT.tile([P, KC, N], FP8, tag=f"{tname}T8",
                            name=f"{tname}T8_{it}")

        def drain_cs(vtag):
            """cs row [1, N] -> SBUF staging (DVE) -> 8 tiny K=1 PE
            transposes -> [P, NT] slice of csp_all.  Zero DMA involvement."""
            row = CS_ROW[vtag]
            sb = cssb.tile([1, N], F32, tag="cs_sb", name=f"sb_{vtag}")
            for nh in range(2):
                nc.vector.tensor_copy(sb[:, nh * HB:(nh + 1) * HB],
                                      cs_all[row:row + 1, nh * HB:(nh + 1) * HB])
            csp = csp_all[:, CSP_IDX[vtag] * NT:(CSP_IDX[vtag] + 1) * NT]
            for j in range(NT):
                nc.tensor.transpose(csp[:, j:j + 1],
                                    sb[0:1, j * P:(j + 1) * P],
                                    ones_f[0:1, 0:1])
            return csp

        # sym-phase row-block order: alternate between the ps1 pool (mt>=4,
        # one bank) and ps2 (mt<4, two banks) so multiple row-blocks are in
        # flight and the PE stream stays dense.
        SYM_ORDER = [4, 0, 5, 1, 6, 2, 3, 7]
        # last contributing mt per colsum half, in emission order
        _contrib = {nh: [mt for mt in SYM_ORDER
                         if max(mt * P + P, nh * HB) < min((nh + 1) * HB, N)]
                    for nh in range(2)}

        def sym_phase(oT, vtag, bridge, warm=0, last=False):
            """Upper-triangle similarity phase: returns rowsums [P, NT]."""
            row = CS_ROW[vtag]
            cs = cs_all[row:row + 1, :]
            rs = stat.tile([P, NT], F32, tag=f"rs{vtag[:2]}", name=f"rs{vtag}")
            # cols [0, P) are never written by any strictly-upper colsum but
            # ARE read by the drain: one tiny zeroing matmul covers them.
            # All other columns get start=True on their first contributor
            # (tracked via cover), so the full-row zeroing openers are gone.
            nc.tensor.matmul(cs[:, 0:P], zeros_b[:], bridge[:, 0:P],
                             start=True, stop=True)
            for w in range(warm):
                nc.tensor.matmul(warm_ps[:], zeros_b[:], bridge[:, 0:HB],
                                 start=True, stop=True)
            pending = []  # colsum matmuls deferred by a few strips

            def flush(n):
                while len(pending) > n:
                    for (dst, lhs, src, start, stop) in pending.pop(0):
                        nc.tensor.matmul(dst, lhs, src, start=start, stop=stop)

            for mt in SYM_ORDER:
                lo = mt * P
                if lo < HB:
                    ps = ps2.tile([P, N], F32, tag="ps2", name="ps_sym")
                    base = 0
                    chunks = [(lo, HB), (HB, N)]
                else:
                    ps = ps1.tile([P, HB], F32, tag="ps1", name="ps_sym1")
                    base = HB
                    chunks = [(lo, N)]
                for ci, (c0, c1) in enumerate(chunks):
                    nc.tensor.matmul(
                        ps[:, c0 - base:c1 - base],
                        oT[:, :, mt * P:(mt + 1) * P],
                        oT[:, :, c0:c1],
                        start=True, stop=(ci > 0),
                        perf_mode=PM.DoubleRow)
                # diag mask: add -1e5*I to [lo, lo+P) inside the group
                nc.tensor.matmul(
                    ps[:, lo - base:lo - base + P], idt[:], negid[:],
                    start=False, stop=True)
                flush(2)
                ee = eep.tile([P, N], BF16, tag="ee", name="ee_sym")
                if N - lo <= 384:
                    # ACT's accum costs a fixed 187ns read; a narrow DVE
                    # reduce is cheaper and ACT is the bottleneck engine
                    nc.scalar.activation(ee[:, lo:], ps[:, lo - base:], AF.Exp)
                    nc.vector.reduce_sum(rs[:, mt:mt + 1], ee[:, lo:],
                                         axis=AX.X)
                else:
                    nc.scalar.activation(ee[:, lo:], ps[:, lo - base:],
                                         AF.Exp, accum_out=rs[:, mt:mt + 1])
                # strictly-upper colsums (lower-triangle rowsums by symmetry)
                # start=True marks the whole per-partition 2KB zero region
                # pending and a write may not straddle the pending/cleared
                # boundary, so: the first emitted strip (mt 4) opens bank 1
                # clearing [640, 1024); strip 0 opens bank 0 and splits its
                # bank-1 write at 640 (left piece overwrites pending bytes,
                # right piece accumulates); everything later is start=False
                # over already-cleared bytes.
                grp = []
                for nh in range(2):
                    a = max(lo + P, nh * HB)
                    b = min((nh + 1) * HB, N)
                    if a >= b:
                        continue
                    stop = mt == _contrib[nh][-1]
                    sp = SYM_ORDER[0] * P + P
                    if mt == 0 and nh == 1 and a < sp < b:
                        grp.append((cs[:, a:sp], ones_b[:], ee[:, a:sp],
                                    False, stop))
                        grp.append((cs[:, sp:b], ones_b[:], ee[:, sp:b],
                                    False, stop))
                    else:
                        start = (mt == SYM_ORDER[0] and nh == 1) or (
                            mt == 0 and nh == 0)
                        grp.append((cs[:, a:b], ones_b[:], ee[:, a:b],
                                    start, stop))
                pending.append(grp)
                # tail phase: emit every pending colsum as soon as the last
                # contributor's exp is queued, so the cs drain chain (copy ->
                # transposes -> ln -> out) starts ~1.5us earlier
                if last and mt == _contrib[1][-1]:
                    flush(0)
            flush(0)
            return rs

        def xy_phase(vtag, xT, yT, bridge):
            row = CS_ROW[vtag]
            cs = cs_all[row:row + 1, :]
            rs = stat.tile([P, NT], F32, tag="rs_xy", name=f"rs_{vtag}")
            pending = []

            def flush(n):
                while len(pending) > n:
                    for (dst, lhs, src, start, stop) in pending.pop(0):
                        nc.tensor.matmul(dst, lhs, src, start=start, stop=stop)

            for mt in range(NT):
                ps = ps2.tile([P, N], F32, tag="ps2", name="ps_xy")
                for nh in range(2):
                    nc.tensor.matmul(
                        ps[:, nh * HB:(nh + 1) * HB],
                        xT[:, :, mt * P:(mt + 1) * P],
                        yT[:, :, nh * HB:(nh + 1) * HB],
                        start=True, stop=True,
                        perf_mode=PM.DoubleRow)
                flush(2)
                ee = eep.tile([P, N], BF16, tag="ee", name="ee_xy")
                nc.scalar.activation(ee[:], ps[:], AF.Exp,
                                     accum_out=rs[:, mt:mt + 1])
                pending.append([
                    (cs[:, nh * HB:(nh + 1) * HB], ones_b[:],
                     ee[:, nh * HB:(nh + 1) * HB], mt == 0, mt == NT - 1)
                    for nh in range(2)])
            flush(0)
            return rs

        def pos_diag(it, xb, yb):
            pos = stat.tile([P, NT], F32, tag="pos", name=f"pos{it}")
            for mt in range(NT):
                pq = scr.tile([P, D], BF16, tag="pq", name="pq")
                nc.vector.scalar_tensor_tensor(
                    pq[:], xb[:, mt * D:(mt + 1) * D], 1.0,
                    yb[:, mt * D:(mt + 1) * D], ALU.mult, ALU.mult,
                    accum_out=pos[:, mt:mt + 1])
            return pos

        def combine_x(it, rs_xx, csT_xx, rs_xy):
            tx = stat.tile([P, NT], F32, tag="tx", name=f"tx{it}")
            nc.vector.tensor_add(tx[:], rs_xy[:], rs_xx[:])
            nc.vector.tensor_add(tx[:], tx[:], csT_xx[:])
            lnx = scr.tile([P, NT], F32, tag="lnx", name="lnx")
            nc.scalar.activation(lnx[:], tx[:], AF.Ln,
                                 accum_out=fin[:, 3 * it:3 * it + 1])

        def combine_y(it, rs_yy, csT_xy, csT_yy):
            ty = stat.tile([P, NT], F32, tag="ty", name=f"ty{it}")
            nc.vector.tensor_add(ty[:], rs_yy[:], csT_xy[:])
            nc.vector.tensor_add(ty[:], ty[:], csT_yy[:])
            lny = scr.tile([P, NT], F32, tag="lny", name="lny")
            nc.scalar.activation(lny[:], ty[:], AF.Ln,
                                 accum_out=fin[:, 3 * it + 1:3 * it + 2])

        def combine_p(it, pos):
            posr = stat.tile([P, 1], F32, tag="posr", name=f"posr{it}")
            nc.vector.reduce_sum(posr[:], pos[:], axis=AX.X)
            nc.vector.tensor_scalar_mul(fin[:, 3 * it + 2:3 * it + 3],
                                        posr[:], -2.0)

        drainA = lambda dst, src: nc.scalar.copy(dst, src)
        drainV = lambda dst, src: nc.vector.tensor_copy(dst, src)

        # ---- item-0 prep: x fully first (xx0 gates on it), y's arithmetic
        # before xx0 (runs on DVE/ACT during xx0), y's PE transposes after
        # xx0's matmuls in the PE FIFO.
        ssx0 = stat.tile([P, NT], F32, tag="ssx", name="ssx0")
        invx0 = stat.tile([P, NT], F32, tag="invx", name="invx0")
        xb0 = xbp.tile([P, NT * D], BF16, tag="xb", name="xb0")
        xT0 = mk_op("x", 0)
        ssy0 = stat.tile([P, NT], F32, tag="ssy", name="ssy0")
        invy0 = stat.tile([P, NT], F32, tag="invy", name="invy0")
        yb0 = xbp.tile([P, NT * D], BF16, tag="yb", name="yb0")
        yT0 = mk_op("y", 0)
        for h in (1, 0):
            prep_arith("x", 0, nx0, h, ssx0, invx0, xb0)
            prep_pe_T("x", 0, xb0, h, xT0, drainA)
        for h in range(2):
            prep_arith("y", 0, ny0, h, ssy0, invy0, yb0)

        # ---- software-pipelined phase emission
        rs_xx0 = sym_phase(xT0, "xx0", xb0, warm=4)
        for h in range(2):
            prep_pe_T("y", 0, yb0, h, yT0, drainA)

        rs_xy0 = xy_phase("xy0", xT0, yT0, xb0)

        # item-1 prep: arithmetic + PE transposes (drained on DVE; ScalarE
        # is the busy engine mid-kernel)
        def prep_full(tname, it, nt_, b, ts):
            """item-1 prep: one merged norm chain."""
            ss = stat.tile([P, NT], F32, tag=f"ss{tname}", name=f"ss{tname}{it}")
            inv10 = stat.tile([P, NT], F32, tag=f"inv{tname}",
                              name=f"inv{tname}{it}")
            for mt in range(NT):
                sq = scr.tile([P, D], BF16, tag="sq", name="sq")
                nc.vector.scalar_tensor_tensor(
                    sq[:], nt_[:, mt * D:(mt + 1) * D], 1.0,
                    nt_[:, mt * D:(mt + 1) * D], ALU.mult, ALU.mult,
                    accum_out=ss[:, mt:mt + 1])
            lns = scr.tile([P, NT], F32, tag="lns", name="lns")
            nc.scalar.activation(lns[:], ss[:], AF.Ln)
            nc.scalar.activation(inv10[:], lns[:], AF.Exp,
                                 scale=-0.5, bias=ln10c[:])
            for mt in range(NT):
                nc.vector.tensor_scalar(
                    b[:, mt * D:(mt + 1) * D], nt_[:, mt * D:(mt + 1) * D],
                    inv10[:, mt:mt + 1], None, ALU.mult)
            # DMA-XBAR transposes (bf16, Sync HWDGE) + one [P, N] fp8
            # cast per k-chunk on DVE: keeps the PE free for phase matmuls
            bd = dram.tile([N, D], BF16, tag=f"{tname}bd", name=f"{tname}bd{it}")
            nc.sync.dma_start(
                bd[:].rearrange("(m p) d -> p m d", p=P),
                b[:].rearrange("p (m d) -> p m d", m=NT))
            for k in range(KC):
                tb = opT.tile([P, N], BF16, tag=f"{tname}Tb{k}",
                              name=f"{tname}Tb{k}_{it}")
                nc.sync.dma_start_transpose(tb[:], bd[:, k * P:(k + 1) * P])
                nc.vector.tensor_copy(ts[:, k, :], tb[:])

        xb1 = xbp.tile([P, NT * D], BF16, tag="xb", name="xb1")
        xT1 = mk_op("x", 1)
        prep_full("x", 1, nx1, xb1, xT1)
        csT_xx0 = drain_cs("xx0")

        yb1 = xbp.tile([P, NT * D], BF16, tag="yb", name="yb1")
        yT1 = mk_op("y", 1)
        prep_full("y", 1, ny1, yb1, yT1)
        csT_xy0 = drain_cs("xy0")

        rs_yy0 = sym_phase(yT0, "yy0", yb0)
        csT_yy0 = drain_cs("yy0")
        pos0 = pos_diag(0, xb0, yb0)

        rs_xx1 = sym_phase(xT1, "xx1", xb1)
        csT_xx1 = drain_cs("xx1")
        combine_x(0, rs_xx0, csT_xx0, rs_xy0)
        combine_y(0, rs_yy0, csT_xy0, csT_yy0)
        combine_p(0, pos0)

        rs_xy1 = xy_phase("xy1", xT1, yT1, xb1)
        csT_xy1 = drain_cs("xy1")
        pos1 = pos_diag(1, xb1, yb1)
        combine_p(1, pos1)

        rs_yy1 = sym_phase(yT1, "yy1", yb1, last=True)
        combine_x(1, rs_xx1, csT_xx1, rs_xy1)
        csT_yy1 = drain_cs("yy1")
        combine_y(1, rs_yy1, csT_xy1, csT_yy1)

        # ship the [128, 6] partials raw; the host does the final reduction.
        # Sync HWDGE: its queue is idle by now and its teardown is ~ns,
        # where the gpsimd SWDGE drain cost ~2.5us after the last op.
        nc.sync.dma_start(out_d[:], fin[:])

    nc.compile()
    return nc


_CACHE = {}
TRACE = False
LAST_RESULTS = None


def _get_nc():
    if "nc" not in _CACHE:
        _CACHE["nc"] = build_nc()
    return _CACHE["nc"]


def make_in_maps(x, y):
    import ml_dtypes

    x = np.ascontiguousarray(np.asarray(x, dtype=np.float32))
    y = np.ascontiguousarray(np.asarray(y, dtype=np.float32))
    idt = np.eye(P, dtype=np.float32).astype(ml_dtypes.bfloat16)
    negid = (np.eye(P, dtype=np.float32) * NEG).astype(ml_dtypes.bfloat16)
    in_maps = []
    for c in range(NCORES):
        in_maps.append({
            "x": x[c * IPC:(c + 1) * IPC].reshape(IPC * N, D),
            "y": y[c * IPC:(c + 1) * IPC].reshape(IPC * N, D),
            "idt": idt,
            "negid": negid,
        })
    return in_maps


def kernel(x, y):
    global LAST_RESULTS
    nc = _get_nc()
    in_maps = make_in_maps(x, y)
    res = run_bass_kernel_spmd(nc, in_maps, list(range(NCORES)), trace=TRACE)
    LAST_RESULTS = res
    partials = np.array([np.asarray(r["out"], dtype=np.float64).sum()
                         for r in res.results])
    return np.float32(partials.sum() / (BS * 2 * N))
